# revision 8
# baseline (speedup 1.0000x reference)
"""DemandMap (histogram_binning) Trainium2 Bass kernel.

Problem (hardcoded from the reference):
  W = H = 2048 site grid, NBX = NBY = 2048 bins -> binW = binH = 1.0.
  Sites sit at integer (r, c); all site types have sx = 1.0, so each
  site contributes ONLY to bin row i = r.  Along c:
    type 1 (sy=1.0):  cap1[r,c] = m1[r,c]
    type 2 (sy=2.5):  cap2[r,c] = m2[r,c] + m2[r,c-1] + 0.5*m2[r,c-2]
    type 3 (sy=5.0):  cap3[r,c] = sum_{k=0..4} m3[r,c-k]
  Output tuple: (1-cap1, 1-cap1, 1-cap2, 1-cap3), binArea = 1.0.

Sharding: rows r split evenly over 8 cores (no halo, no collectives).

Layout: each core's 256-row slab lives as TWO 128-row "islands" on the
128 SBUF partitions: sbuf [128 part, 2 isl, 2048 cols]; the matching
DRAM tensors are [128, 2, 2048] (host pre-permutes).  Elementwise ops
cover both islands at once (free size = 2*cols) — half the instruction
count of a 2-tile scheme.  Bordered buffers ([128, 2, C+b]) carry
per-island zero left-borders so out-of-range taps vanish.

Engine split (driven by the TimelineSim cost model):
  DVE : masks m2/m3 ((x==t), tensor_scalar 4x perf mode), o0=(x!=1)
        (4x, bf16), a2=m2+s1(m2), a3=m3+s1(m3), b3=a3+s2(a3),
        o3=g3-b3 (tensor_tensor 2x, bf16) — chunked so stores drain
        while later chunks compute.
  ACT : h2 = 1-0.5*s2(m2), g3 = 1-s4(m3) (Copy activation, the "+1"
        rides the bias); o2 store issues.
  POOL: border memsets; o2 = h2-a2 (tensor_tensor) straight to fp8e4
        (values are halves in [-1.5,1]: exact).
  SP  : chunked loads (small first chunk so DVE starts early) +
        o0/o3 stores in production order.
"""

from contextlib import ExitStack

import numpy as np
import ml_dtypes

import concourse.bass as bass
import concourse.mybir as mybir
from concourse.bass_utils import run_bass_kernel_spmd

N_CORES = 8
W = 2048               # rows r (site x / bin x)
C = 2048               # cols c (site y / bin y)
R_PER = W // N_CORES   # 256 rows per core
P = 128                # SBUF partitions
NI = R_PER // P        # 2 islands per core

# load / mask chunks (small first chunk -> first compute ASAP)
LCH = [(0, 384), (384, 1024), (1024, 2048)]
# half-split used by a2/a3/b3/h2/g3
HCH = [(0, 1024), (1024, 2048)]
# o3 compute/store quarters (tiny last chunk); o2 store thirds (fp8)
Q3 = [(0, 512), (512, 1024), (1024, 1792), (1792, 2048)]
Q2 = [(0, 1024), (1024, 1792), (1792, 2048)]

_A = mybir.AluOpType
BF = mybir.dt.bfloat16
F8 = mybir.dt.float8e4

LAST_RESULTS = None  # BassKernelResults of the most recent run (for test.py)


def _build_program():
    nc = bass.Bass()
    stm = nc.dram_tensor("stm", [P, NI, C], BF, kind="ExternalInput")
    o0d = nc.dram_tensor("o0", [P, NI, C], BF, kind="ExternalOutput")
    o2d = nc.dram_tensor("o2", [P, NI, C], F8, kind="ExternalOutput")
    o3d = nc.dram_tensor("o3", [P, NI, C], BF, kind="ExternalOutput")

    with ExitStack() as ctx:
        sb = lambda nm, w, dt=BF: ctx.enter_context(
            nc.sbuf_tensor(nm, [P, NI, w], dt))
        X = sb("X", C)
        M2 = sb("M2", C + 2)    # data @2, zero border 0:2 (shifts 1,2)
        M3 = sb("M3", C + 4)    # data @4, zero border 0:4 (shifts 1,4)
        A2 = sb("A2", C)
        A3 = sb("A3", C + 2)    # data @2, zero border 0:2 (shift 2)
        B3 = sb("B3", C)
        H2 = sb("H2", C)
        G3 = sb("G3", C)
        O0 = sb("O0", C)
        O2 = sb("O2", C, F8)
        O3 = sb("O3", C)

        sem = lambda nm: ctx.enter_context(nc.semaphore(nm))
        in_s = [sem(f"in{i}_s") for i in range(len(LCH))]
        ms_s = sem("ms_s")
        m2_s, m3_s = sem("m2_s"), sem("m3_s")
        h2_s, a2_s, a3_s, b3_s, g3_s = (sem("h2_s"), sem("a2_s"),
                                        sem("a3_s"), sem("b3_s"), sem("g3_s"))
        o0_s, o2_s, o3_s = sem("o0_s"), sem("o2_s"), sem("o3_s")
        out0_s, out2_s, out3_s = sem("out0_s"), sem("out2_s"), sem("out3_s")
        block = ctx.enter_context(nc.Block())

        # shifted data views of bordered buffers: data sits at col offset b;
        # tap k of col range [lo:hi) reads cols b+lo-k : b+hi-k.
        def v(t, b, k, lo, hi):
            return t[:, :, b + lo - k : b + hi - k]

        @block.sync
        def _(sync):
            for i, (lo, hi) in enumerate(LCH):
                sync.dma_start(out=X[:, :, lo:hi], in_=stm[:, :, lo:hi]
                               ).then_inc(in_s[i], 16)
            for qi, (lo, hi) in enumerate(Q3):
                sync.dma_start(out=o3d[:, :, lo:hi], in_=O3[:, :, lo:hi]
                               )._wait_ge(o3_s, qi + 1).then_inc(out3_s, 16)
            sync.wait_ge(out3_s, 16 * len(Q3))

        @block.gpsimd
        def _(gp):
            # GpSimd completion is NOT in program order: consumers of the
            # border zeros wait on completion-attached increments.
            gp.memset(M2[:, :, 0:2], 0.0).then_inc(ms_s, 1)
            gp.memset(M3[:, :, 0:4], 0.0).then_inc(ms_s, 1)
            gp.memset(A3[:, :, 0:2], 0.0).then_inc(ms_s, 1)
            # o2 = h2 - a2, straight to fp8 (engine is dtype-blind).
            # chunk qi needs h2/a2 over [lo:hi): h2 halves 1..2, a2 halves.
            for qi, (lo, hi) in enumerate(Q2):
                h_need = 1 if hi <= 1024 else 2
                gp.wait_ge(h2_s, h_need)
                gp.tensor_tensor(
                    O2[:, :, lo:hi], H2[:, :, lo:hi], A2[:, :, lo:hi],
                    _A.subtract,
                )._wait_ge(a2_s, h_need).then_inc(o2_s, 1)

        @block.scalar
        def _(act):
            Copy = mybir.ActivationFunctionType.Copy
            # h2 = 1 - 0.5*s2(m2), g3 = 1 - s4(m3); chunk [lo:hi) reads mask
            # data cols lo-k..hi-k (border + load chunks covering hi).
            act.wait_ge(ms_s, 3)
            act.activation(H2[:, :, 0:1024], v(M2, 2, 2, 0, 1024), Copy,
                           bias=1.0, scale=-0.5
                           )._wait_ge(m2_s, 2).then_inc(h2_s, 1)
            act.activation(G3[:, :, 0:1024], v(M3, 4, 4, 0, 1024), Copy,
                           bias=1.0, scale=-1.0
                           )._wait_ge(m3_s, 2).then_inc(g3_s, 1)
            act.activation(H2[:, :, 1024:C], v(M2, 2, 2, 1024, C), Copy,
                           bias=1.0, scale=-0.5
                           )._wait_ge(m2_s, 3).then_inc(h2_s, 1)
            act.activation(G3[:, :, 1024:C], v(M3, 4, 4, 1024, C), Copy,
                           bias=1.0, scale=-1.0
                           )._wait_ge(m3_s, 3).then_inc(g3_s, 1)
            # store issues ride the idle ACT SEQ, in production order
            for lo, hi in HCH:
                act.dma_start(out=o0d[:, :, lo:hi], in_=O0[:, :, lo:hi]
                              )._wait_ge(o0_s, 1).then_inc(out0_s, 16)
            for qi in range(len(Q2)):
                lo, hi = Q2[qi]
                act.dma_start(out=o2d[:, :, lo:hi], in_=O2[:, :, lo:hi]
                              )._wait_ge(o2_s, qi + 1).then_inc(out2_s, 16)
            act.wait_ge(out0_s, 32)
            act.wait_ge(out2_s, 16 * len(Q2))

        @block.vector
        def _(vec):
            # Producers all carry then_inc (which skips the race model's
            # implicit program-order chain), so every RAW is threaded through
            # explicit sems attached to the consuming instruction.
            ts, tt = vec.tensor_scalar, vec.tensor_tensor
            # masks follow load chunks; m3 first (deeper chain).
            for i, (lo, hi) in enumerate(LCH[:2]):
                ts(v(M3, 4, 0, lo, hi), X[:, :, lo:hi], 3, None,
                   _A.is_equal)._wait_ge(in_s[i], 16).then_inc(m3_s, 1)
                ts(v(M2, 2, 0, lo, hi), X[:, :, lo:hi], 2, None,
                   _A.is_equal)._wait_ge(in_s[i], 16).then_inc(m2_s, 1)
            # first halves of the chains (cols 0:1024 need mask chunks 0-1)
            vec.wait_ge(ms_s, 3)
            tt(A2[:, :, 0:1024], v(M2, 2, 0, 0, 1024), v(M2, 2, 1, 0, 1024),
               _A.add)._wait_ge(m2_s, 2).then_inc(a2_s, 1)
            tt(v(A3, 2, 0, 0, 1024), v(M3, 4, 0, 0, 1024),
               v(M3, 4, 1, 0, 1024), _A.add
               )._wait_ge(m3_s, 2).then_inc(a3_s, 1)
            # last mask chunk
            lo, hi = LCH[2]
            ts(v(M3, 4, 0, lo, hi), X[:, :, lo:hi], 3, None,
               _A.is_equal)._wait_ge(in_s[2], 16).then_inc(m3_s, 1)
            ts(v(M2, 2, 0, lo, hi), X[:, :, lo:hi], 2, None,
               _A.is_equal)._wait_ge(in_s[2], 16).then_inc(m2_s, 1)
            # o0 mid-stream so its (big, bf16) store drains during compute
            ts(O0[:, :, :], X[:, :, :], 1, None, _A.not_equal
               )._wait_ge(in_s[2], 16).then_inc(o0_s, 1)
            tt(A2[:, :, 1024:C], v(M2, 2, 0, 1024, C), v(M2, 2, 1, 1024, C),
               _A.add)._wait_ge(m2_s, 3).then_inc(a2_s, 1)
            tt(v(A3, 2, 0, 1024, C), v(M3, 4, 0, 1024, C),
               v(M3, 4, 1, 1024, C), _A.add
               )._wait_ge(m3_s, 3).then_inc(a3_s, 1)
            tt(B3[:, :, 0:1024], v(A3, 2, 0, 0, 1024), v(A3, 2, 2, 0, 1024),
               _A.add)._wait_ge(a3_s, 1).then_inc(b3_s, 1)
            # o3 quarters 0-1 (need b3 half 0 + g3 half 0)
            for qi in (0, 1):
                lo, hi = Q3[qi]
                vec.wait_ge(g3_s, 1)
                tt(O3[:, :, lo:hi], G3[:, :, lo:hi], B3[:, :, lo:hi],
                   _A.subtract)._wait_ge(b3_s, 1).then_inc(o3_s, 1)
            tt(B3[:, :, 1024:C], v(A3, 2, 0, 1024, C), v(A3, 2, 2, 1024, C),
               _A.add)._wait_ge(a3_s, 2).then_inc(b3_s, 1)
            for qi in (2, 3):
                lo, hi = Q3[qi]
                vec.wait_ge(g3_s, 2)
                tt(O3[:, :, lo:hi], G3[:, :, lo:hi], B3[:, :, lo:hi],
                   _A.subtract)._wait_ge(b3_s, 2).then_inc(o3_s, 1)

    return nc


def kernel(site_type_map, node_size_x, node_size_y, width, height,
           num_bins_x, num_bins_y, xl, xh, yl, yh):
    global LAST_RESULTS
    stm = np.asarray(site_type_map, dtype=np.int32).reshape(W, C)
    stm_bf = stm.astype(ml_dtypes.bfloat16)  # values 0..3: exact in bf16

    nc = _build_program()
    in_maps = []
    for k in range(N_CORES):
        slab = stm_bf[k * R_PER:(k + 1) * R_PER, :]
        # [256, 2048] -> [128 part, 2 isl, 2048]; island i holds rows i*128+p
        arr = np.ascontiguousarray(slab.reshape(NI, P, C).transpose(1, 0, 2))
        in_maps.append({"stm": arr})
    res = run_bass_kernel_spmd(nc, in_maps, core_ids=list(range(N_CORES)))
    LAST_RESULTS = res

    def gather(name):
        slabs = []
        for k in range(N_CORES):
            arr = np.asarray(res.results[k][name]).astype(np.float32)
            slabs.append(arr.transpose(1, 0, 2).reshape(R_PER, C))
        return np.concatenate(slabs, axis=0)

    out0 = gather("o0")
    out2 = gather("o2")
    out3 = gather("o3")
    return (out0, out0, out2, out3)


# revision 10
# speedup vs baseline: 1.0729x; 1.0729x over previous
"""DemandMap (histogram_binning) Trainium2 Bass kernel — PE-conv design.

Problem (hardcoded from the reference):
  W = H = 2048 site grid, NBX = NBY = 2048 bins -> binW = binH = 1.0.
  Sites sit at integer (r, c); all site types have sx = 1.0, so each
  site contributes ONLY to bin row i = r.  Along c:
    type 1 (sy=1.0):  cap1[r,c] = m1[r,c]
    type 2 (sy=2.5):  cap2[r,c] = m2[r,c] + m2[r,c-1] + 0.5*m2[r,c-2]
    type 3 (sy=5.0):  cap3[r,c] = sum_{k=0..4} m3[r,c-k]
  Output tuple: (1-cap1, 1-cap1, 1-cap2, 1-cap3), binArea = 1.0.

Layout: TRANSPOSED — c on partitions, r on the free axis (host supplies
xT).  Core k owns c in [k*256, (k+1)*256) as two 128-row islands
[128 part, 2 isl, 2048 r].  The window sums along c are then
partition-direction convolutions, which the (otherwise idle) PE engine
computes as band-matrix matmuls into PSUM:

    cap[po, r] = sum_k W[k, po] * m[k, r]   (W upper-band Toeplitz)

Cross-boundary taps (first 4 output rows of each island) accumulate via
a second K=4 "halo" matmul: island 0 reads a 4-row halo tile shipped
from the neighbouring core's range (zeros for core 0); island 1 reads
partitions 124:128 of island 0's own mask.  The "1 - cap" fold rides
the PSUM eviction (Copy activation / tensor_scalar, scale=-1 bias=1),
which also downcasts straight to fp8e4 (all values are halves in
[-4, 1]: exact).  PE is kept at full clock by warm-up matmuls on a
scratch buffer while the input streams in.

Engine split:
  PE  : 10 warm-up matmuls + 32 band/halo matmuls (2 maps x 2 islands
        x 2 r-halves x 2 chunks x {main, halo}), double-buffered PSUM.
  DVE : masks m2/m3 ((x==t) tensor_scalar, 4x perf mode, chunked after
        the loads), halo masks, o0=(x!=1) (4x, bf16), map-3 evictions.
  ACT : map-2 evictions; halo load + o2 store issues.
  POOL: warm-up scratch memset only.
  SP  : weight + input loads, o0/o3 stores.

Weights travel inside the input tensors (cols 2048:2176 of stm_ext,
cols 2048:2304 of the halo tile) so no extra DMA slots sit on the
critical early HWDGE path.
"""

from contextlib import ExitStack

import numpy as np
import ml_dtypes

import concourse.bass as bass
import concourse.mybir as mybir
from concourse.bass_utils import run_bass_kernel_spmd

N_CORES = 8
GRID = 2048
C_PER = 256            # c rows per core
P = 128                # SBUF partitions
NI = 2                 # islands per core
R = 2048               # free axis (r)
WCOL = 128             # weight columns appended per island
XW = R + WCOL          # stm_ext free width

# r-chunks for loads/masks (small first chunk -> first compute ASAP)
LCH = [(0, 384), (384, 1024), (1024, 2048)]
N_WARMUP = 10

_A = mybir.AluOpType
BF = mybir.dt.bfloat16
F8 = mybir.dt.float8e4
F32 = mybir.dt.float32

LAST_RESULTS = None  # BassKernelResults of the most recent run (for test.py)

# fills: (map, island, r_half); r-half 0 fills first (need only load
# chunks 0-1), so PE starts long before the last chunk lands.
FILLS = [(2, 0, 0), (3, 0, 0), (2, 1, 0), (3, 1, 0),
         (2, 0, 1), (3, 0, 1), (2, 1, 1), (3, 1, 1)]


def _weights():
    """Band matrices: main W[k, m] = w(m-k); halo Wh[j, m] = w(m-j+4)."""
    w2 = {0: 1.0, 1: 1.0, 2: 0.5}
    w3 = {0: 1.0, 1: 1.0, 2: 1.0, 3: 1.0, 4: 1.0}
    def band(w, rows, off):
        a = np.zeros((rows, P), np.float32)
        for k in range(rows):
            for m in range(P):
                d = m - k + off
                if d in w:
                    a[k, m] = w[d]
        return a
    return (band(w2, P, 0), band(w2, 4, 4), band(w3, P, 0), band(w3, 4, 4))


def _build_program():
    nc = bass.Bass()
    stm = nc.dram_tensor("stm", [P, NI, XW], BF, kind="ExternalInput")
    xhd = nc.dram_tensor("xh", [36, R + 2 * WCOL], BF, kind="ExternalInput")
    o0d = nc.dram_tensor("o0", [P, NI, R], BF, kind="ExternalOutput")
    o2d = nc.dram_tensor("o2", [P, NI, R], F8, kind="ExternalOutput")
    o3d = nc.dram_tensor("o3", [P, NI, R], F8, kind="ExternalOutput")

    with ExitStack() as ctx:
        X = ctx.enter_context(nc.sbuf_tensor("X", [P, NI, XW], BF))
        XH = ctx.enter_context(nc.sbuf_tensor("XH", [36, R + 2 * WCOL], BF))
        M2 = ctx.enter_context(nc.sbuf_tensor("M2", [P, NI, R], BF))
        M3 = ctx.enter_context(nc.sbuf_tensor("M3", [P, NI, R], BF))
        M2H = ctx.enter_context(nc.sbuf_tensor("M2H", [36, R], BF))
        M3H = ctx.enter_context(nc.sbuf_tensor("M3H", [36, R], BF))
        WUP = ctx.enter_context(nc.sbuf_tensor("WUP", [P, 512], BF))
        O0 = ctx.enter_context(nc.sbuf_tensor("O0", [P, NI, R], BF))
        O2 = ctx.enter_context(nc.sbuf_tensor("O2", [P, NI, R], F8))
        O3 = ctx.enter_context(nc.sbuf_tensor("O3", [P, NI, R], F8))
        PS2 = ctx.enter_context(nc.psum_tensor("PS2", [P, 2, 1024], F32))
        PS3 = ctx.enter_context(nc.psum_tensor("PS3", [P, 2, 1024], F32))

        sem = lambda nm: ctx.enter_context(nc.semaphore(nm))
        lw_s = sem("lw_s")
        in_s = [sem(f"in{i}_s") for i in range(len(LCH))]
        ld_iss = sem("ld_iss")
        lh_s = sem("lh_s")
        wup_s = sem("wup_s")
        m2_s, m3_s, mh_s = sem("m2_s"), sem("m3_s"), sem("mh_s")
        f2_s, f3_s = sem("f2_s"), sem("f3_s")
        ev2_s, ev3_s = sem("ev2_s"), sem("ev3_s")
        o0_s = sem("o0_s")
        out0_s, out2_s, out3_s = sem("out0_s"), sem("out2_s"), sem("out3_s")
        block = ctx.enter_context(nc.Block())

        # weight views (tails of the input tensors)
        W2M = X[:, 0, R:XW]
        W3M = X[:, 1, R:XW]
        # halo weights duplicated at partitions 0 and 32 (PE requires
        # lhsT/rhs base partition in {0, 32, 64}; island 1's halo rows sit
        # at partition 32)
        W2H = [XH[0:4, R:R + WCOL], XH[32:36, R:R + WCOL]]
        W3H = [XH[0:4, R + WCOL:R + 2 * WCOL],
               XH[32:36, R + WCOL:R + 2 * WCOL]]
        MS = {2: M2, 3: M3}
        MHS = {2: M2H, 3: M3H}
        PSS = {2: PS2, 3: PS3}
        FS = {2: f2_s, 3: f3_s}
        EVS = {2: ev2_s, 3: ev3_s}

        @block.sync
        def _(sync):
            sync.dma_start(out=X[:, :, R:XW], in_=stm[:, :, R:XW]
                           ).then_inc(lw_s, 16)
            for i, (lo, hi) in enumerate(LCH):
                sync.dma_start(out=X[:, :, lo:hi], in_=stm[:, :, lo:hi]
                               ).then_inc(in_s[i], 16)
            sync.sem_inc(ld_iss, 1)
            for lo, hi in [(0, 1024), (1024, 2048)]:
                sync.dma_start(out=o0d[:, :, lo:hi], in_=O0[:, :, lo:hi]
                               )._wait_ge(o0_s, 1).then_inc(out0_s, 16)
            # o3 stores per eviction (fp8 quarters, tiny transfers)
            for i, (isl, rh) in enumerate([(0, 0), (1, 0), (0, 1), (1, 1)]):
                lo, hi = rh * 1024, (rh + 1) * 1024
                sync.dma_start(out=o3d[:, isl, lo:hi], in_=O3[:, isl, lo:hi]
                               )._wait_ge(ev3_s, i + 1).then_inc(out3_s, 16)
            sync.wait_ge(out0_s, 32)
            sync.wait_ge(out3_s, 64)

        @block.gpsimd
        def _(gp):
            gp.memset(WUP[:, :], 0.0).then_inc(wup_s, 1)

        @block.tensor
        def _(pe):
            # warm-up: PE p-state needs ~3us of continuous busy before the
            # real matmuls; garbage matmuls on the zeroed scratch buffer.
            pe.wait_ge(wup_s, 1)
            pe.wait_ge(lw_s, 16)
            for _i in range(N_WARMUP):
                pe.matmul(PS2[:, 0, 0:512], WUP[:, 0:128], WUP[:, :],
                          start=True, stop=True)
            fcnt = {2: 0, 3: 0}
            for mp, isl, rh in FILLS:
                M, MH, PS = MS[mp], MHS[mp], PSS[mp]
                WM = W2M if mp == 2 else W3M
                WH = W2H if mp == 2 else W3H
                i_m = fcnt[mp]
                fcnt[mp] += 1
                slot = i_m % 2
                # masks for this r-half must be complete
                need = 2 if rh == 0 else 3
                pe.wait_ge(m2_s if mp == 2 else m3_s, need)
                pe.wait_ge(mh_s, 1 if mp == 2 else 2)
                if i_m >= 2:  # PSUM slot reuse: prior eviction must be done
                    pe.wait_ge(EVS[mp], i_m - 1)
                for ch in range(2):
                    lo = rh * 1024 + ch * 512
                    pe.matmul(PS[:, slot, ch * 512:(ch + 1) * 512],
                              WM, M[:, isl, lo:lo + 512],
                              start=True, stop=False)
                for ch in range(2):
                    lo = rh * 1024 + ch * 512
                    halo = (MH[0:4, lo:lo + 512] if isl == 0
                            else MH[32:36, lo:lo + 512])
                    mm = pe.matmul(PS[:, slot, ch * 512:(ch + 1) * 512],
                                   WH[isl], halo, start=False, stop=True)
                    if ch == 1:
                        mm.then_inc(FS[mp], 1)

        @block.scalar
        def _(act):
            Copy = mybir.ActivationFunctionType.Copy
            act.wait_ge(ld_iss, 1)
            act.dma_start(out=XH[:, :], in_=xhd[:, :]).then_inc(lh_s, 16)
            # map-2 evictions: o2 = 1 - psum2, straight to fp8
            for i, (isl, rh) in enumerate([(0, 0), (1, 0), (0, 1), (1, 1)]):
                lo, hi = rh * 1024, (rh + 1) * 1024
                act.activation(O2[:, isl, lo:hi], PS2[:, i % 2, :], Copy,
                               bias=1.0, scale=-1.0
                               )._wait_ge(f2_s, i + 1).then_inc(ev2_s, 1)
                act.dma_start(out=o2d[:, isl, lo:hi], in_=O2[:, isl, lo:hi]
                              )._wait_ge(ev2_s, i + 1).then_inc(out2_s, 16)
            act.wait_ge(out2_s, 64)

        @block.vector
        def _(vec):
            ts = vec.tensor_scalar
            # masks follow the load chunks; m3 first (join partner of the
            # first PE fill is map 2 though, so m2 right behind).
            lo, hi = LCH[0]
            ts(M3[:, :, lo:hi], X[:, :, lo:hi], 3, None, _A.is_equal
               )._wait_ge(in_s[0], 16).then_inc(m3_s, 1)
            ts(M2[:, :, lo:hi], X[:, :, lo:hi], 2, None, _A.is_equal
               )._wait_ge(in_s[0], 16).then_inc(m2_s, 1)
            lo, hi = LCH[1]
            ts(M3[:, :, lo:hi], X[:, :, lo:hi], 3, None, _A.is_equal
               )._wait_ge(in_s[1], 16).then_inc(m3_s, 1)
            ts(M2[:, :, lo:hi], X[:, :, lo:hi], 2, None, _A.is_equal
               )._wait_ge(in_s[1], 16).then_inc(m2_s, 1)
            # halo masks (tiny partitions, full r)
            ts(M2H[:, :], XH[0:36, 0:R], 2, None, _A.is_equal
               )._wait_ge(lh_s, 16).then_inc(mh_s, 1)
            ts(M3H[:, :], XH[0:36, 0:R], 3, None, _A.is_equal
               )._wait_ge(lh_s, 16).then_inc(mh_s, 1)
            lo, hi = LCH[2]
            ts(M3[:, :, lo:hi], X[:, :, lo:hi], 3, None, _A.is_equal
               )._wait_ge(in_s[2], 16).then_inc(m3_s, 1)
            ts(M2[:, :, lo:hi], X[:, :, lo:hi], 2, None, _A.is_equal
               )._wait_ge(in_s[2], 16).then_inc(m2_s, 1)
            # o0 = (x != 1), early so its bf16 store drains mid-pipe
            ts(O0[:, :, :], X[:, :, 0:R], 1, None, _A.not_equal
               )._wait_ge(in_s[2], 16).then_inc(o0_s, 1)
            # map-3 evictions: o3 = 1 - psum3 (PSUM in -> full rate)
            for i, (isl, rh) in enumerate([(0, 0), (1, 0), (0, 1), (1, 1)]):
                lo, hi = rh * 1024, (rh + 1) * 1024
                ts(O3[:, isl, lo:hi], PS3[:, i % 2, :], -1.0, 1.0,
                   _A.mult, op1=_A.add
                   )._wait_ge(f3_s, i + 1).then_inc(ev3_s, 1)

    return nc


def kernel(site_type_map, node_size_x, node_size_y, width, height,
           num_bins_x, num_bins_y, xl, xh, yl, yh):
    global LAST_RESULTS
    stm = np.asarray(site_type_map, dtype=np.int32).reshape(GRID, GRID)
    xT = np.ascontiguousarray(stm.T).astype(ml_dtypes.bfloat16)  # [c, r]

    w2m, w2h, w3m, w3h = _weights()
    wmain = np.concatenate(  # [128, 2, 128]: island 0 = W2M, island 1 = W3M
        [w2m[:, None, :], w3m[:, None, :]], axis=1).astype(ml_dtypes.bfloat16)
    whalo = np.concatenate([w2h, w3h], axis=1).astype(ml_dtypes.bfloat16)

    nc = _build_program()
    in_maps = []
    for k in range(N_CORES):
        slab = xT[k * C_PER:(k + 1) * C_PER, :]          # [256, 2048]
        arr = slab.reshape(NI, P, GRID).transpose(1, 0, 2)  # [128, 2, 2048]
        stm_ext = np.concatenate([arr, wmain], axis=2)   # [128, 2, 2176]
        xh = np.zeros((36, GRID + 2 * WCOL), ml_dtypes.bfloat16)
        if k > 0:
            xh[0:4, 0:GRID] = xT[k * C_PER - 4:k * C_PER, :]
        # island 1's halo = last 4 c-rows of island 0, parked at part 32
        xh[32:36, 0:GRID] = xT[k * C_PER + P - 4:k * C_PER + P, :]
        xh[0:4, GRID:] = whalo
        xh[32:36, GRID:] = whalo
        xh_ext = xh
        in_maps.append({"stm": np.ascontiguousarray(stm_ext),
                        "xh": np.ascontiguousarray(xh_ext)})
    res = run_bass_kernel_spmd(nc, in_maps, core_ids=list(range(N_CORES)))
    LAST_RESULTS = res

    def gather(name):
        slabs = []
        for k in range(N_CORES):
            arr = np.asarray(res.results[k][name]).astype(np.float32)
            slabs.append(arr.transpose(1, 0, 2).reshape(C_PER, GRID))
        return np.ascontiguousarray(np.concatenate(slabs, axis=0).T)

    out0 = gather("o0")
    out2 = gather("o2")
    out3 = gather("o3")
    return (out0, out0, out2, out3)
